# revision 29
# baseline (speedup 1.0000x reference)
"""ContextBasedSumAttention Trainium2 Bass kernel.

Math (per batch row b):
    u[h]      = sum_k h_t[b,k] * W[k,h]                  (h_t @ W)
    scores[s] = sum_h cntx[b,s,h] * u[h]
    attn      = softmax(scores)
    cx[h]     = sum_s attn[s] * cntx[b,s,h]
    out[b]    = alpha * h_t[b] + beta * cx

Sharding: data-parallel over batch across 8 NeuronCores (4 rows each).
W / alpha / beta replicated.

Per-core dataflow (single pass over cntx, natural [s,h] layout):
  setup (all off the gpsimd queue so cb DMA desc-gen starts at once):
  - h_t loaded contiguous [4,1024], transposed on PE (identity matmul)
  - W streamed on the sync HWDGE queue; U = h_tT @ W on PE (fp32)
  - U rows broadcast to 128 partitions via PE selector matmul + ACT copy
  per batch row (cb DMAs emitted one row ahead so SWDGE desc-gen never
  waits on compute):
  - casting DMA (SWDGE gpsimd) cntx[b] fp32 -> SBUF fp16
      [128(p=s%128), 16(t), 1024(h)]
  - phase 1: 16x DVE fused scalar_tensor_tensor (x fp16 u_bc, fp32
      accum over h) -> scores[128,16] fp32   (~1.2 cyc/elem measured)
  - softmax with NO PE in the dependency chain (the PE is busy streaming
      the previous row's phase 2; its in-order queue would otherwise
      delay att by a full phase-2 drain):
      m_p = rowmax (DVE), e = exp(scores-m_p) + row sums l_p (ACT),
      t_p = exp(m_p-128) (ACT), q = l_p*t_p (DVE),
      L -> all partitions via gpsimd partition_all_reduce(add),
      att = e * t_p / L (DVE; exact softmax in (0,1] -> fp16)
  - phase 2: 16x2 PE fp16 matmuls (1 cyc/row), lhsT = att[:,t], rhs = cb
      -> o2 psum[1,1024] fp32
  - out_row = o2 * beta + alpha*h_t[b]: exact fp32 DVE scalar_tensor_tensor,
      deferred TWO rows so the DVE never stalls on phase-2 PE completion;
      out DMA on the scalar queue
Engine queues: gpsimd=cb casting DMA + allreduce; sync=W; scalar=out rows.
Steady state is DMA-bound (~108us/iter floor per core: 32 MiB cntx +
4 MiB W fp32 HBM reads at ~358 GB/s).
"""

from contextlib import ExitStack

import numpy as np

import concourse.bass as bass
import concourse.tile as tile
from concourse import bacc, bass_isa, mybir
from concourse.bass import ds
from concourse.bass_utils import run_bass_kernel_spmd
from concourse.masks import make_identity

N_CORES = 8
B, S, H = 32, 2048, 1024
B_LOC = B // N_CORES      # 4 batch rows per core
P = 128                   # SBUF partitions
T = S // P                # 16 s-tiles
KC = H // P               # 8 k-chunks of W
NHALF = H // 2            # 512 = max fp32 matmul free dim
C_OFF = 128.0             # per-partition softmax offset (fp32-safe range)
F32 = mybir.dt.float32
FP16 = mybir.dt.float16
ALU = mybir.AluOpType
ACTF = mybir.ActivationFunctionType
DMA_GROUPS = 4            # split each 8 MiB cntx read into 4 DMAs


def _emit(ctx, tc, nc, ht, cm, w, al, be, out, skip=()):
    singles = ctx.enter_context(tc.tile_pool(name="singles", bufs=1))
    cpool = ctx.enter_context(tc.tile_pool(name="cpool", bufs=4))
    spool = ctx.enter_context(tc.tile_pool(name="spool", bufs=2))
    small = ctx.enter_context(tc.tile_pool(name="small", bufs=4))
    opool = ctx.enter_context(tc.tile_pool(name="opool", bufs=2))

    # ---- setup: U = h_t @ W on PE, rows broadcast via PE; constants ----
    # gpsimd only sees tiny input DMAs + identity fill before cb DMAs.
    al_sb = singles.tile([1, 1], F32, tag="al_sb")
    nc.gpsimd.dma_start(out=al_sb[:], in_=al[:].unsqueeze(0))
    be_sb = singles.tile([1, 1], F32, tag="be_sb")
    nc.gpsimd.dma_start(out=be_sb[:], in_=be[:].unsqueeze(0))
    # h_t contiguous [4, 1024] (4 descriptors) + flat copy at partition 0
    ht_sb = singles.tile([B_LOC, H], F32, tag="ht_sb")
    nc.gpsimd.dma_start(out=ht_sb[:], in_=ht[:, :])
    aht = singles.tile([1, B_LOC * H], F32, tag="aht")
    nc.gpsimd.dma_start(out=aht[:], in_=ht.rearrange("b h -> (b h)").unsqueeze(0))
    ident = singles.tile([B_LOC, B_LOC], F32, tag="ident")
    make_identity(nc, ident[:])

    # aht = alpha * h_t, flat [1, B_LOC*H] (in-place scale)
    nc.vector.tensor_scalar_mul(aht[:], aht[:], al_sb[:])

    noff = singles.tile([P, 1], F32, tag="noff")
    nc.vector.memset(noff[:], -C_OFF)

    u_bc = []
    with tc.tile_pool(name="wpool", bufs=2) as wpool, tc.tile_pool(
        name="setup", bufs=1
    ) as setup, tc.tile_pool(name="psum_u", bufs=1, space="PSUM") as psum_u, tc.tile_pool(
        name="psum_b", bufs=2, space="PSUM"
    ) as psum_b:
        # ht_t[p, c, b] = h_t[b, c*128+p] via PE identity transpose
        htt_ps = psum_u.tile([P, KC, B_LOC], F32, tag="htt_ps")
        for c in range(KC):
            nc.tensor.transpose(
                htt_ps[:, c, :], ht_sb[:, ds(c * P, P)], ident[:]
            )
        ht_t = setup.tile([P, KC, B_LOC], F32, tag="ht_t")
        nc.scalar.copy(ht_t[:], htt_ps[:])

        u_ps = psum_u.tile([B_LOC, H], F32, tag="u_ps")
        for c in range(KC):
            wt = wpool.tile([P, H], F32, tag="w")
            nc.sync.dma_start(out=wt[:], in_=w[ds(c * P, P), :])
            for nh in range(2):
                nc.tensor.matmul(
                    u_ps[:, ds(nh * NHALF, NHALF)],
                    ht_t[:, c, :],
                    wt[:, ds(nh * NHALF, NHALF)],
                    start=(c == 0),
                    stop=(c == KC - 1),
                )
        u_sb = setup.tile([B_LOC, H], F32, tag="u_sb")
        nc.scalar.copy(u_sb[:], u_ps[:])

        # broadcast each u row to 128 partitions: sel_b.T @ u_sb where
        # sel[k, b, :] = 1 iff k == b (rhs must sit at base partition 0)
        sel = setup.tile([B_LOC, B_LOC, P], F32, tag="sel")
        nc.gpsimd.memset(sel[:], 0.0)
        nc.gpsimd.affine_select(
            out=sel[:],
            in_=sel[:],
            compare_op=ALU.not_equal,
            fill=1.0,
            base=0,
            # iota = k*1 + b*(-1) + i*0; fill 1.0 where iota == 0 (k == b)
            pattern=[[-1, B_LOC], [0, P]],
            channel_multiplier=1,
        )
        for b in range(B_LOC):
            ub_ps = psum_b.tile([P, H], F32, tag="ubps")
            for nh in range(2):
                nc.tensor.matmul(
                    ub_ps[:, ds(nh * NHALF, NHALF)],
                    sel[:, b, :],
                    u_sb[:, ds(nh * NHALF, NHALF)],
                    start=True,
                    stop=True,
                )
            ub = singles.tile([P, H], FP16, tag=f"ubc{b}")
            nc.scalar.copy(ub[:], ub_ps[:])
            u_bc.append(ub)

    # PSUM pool for the row pipeline (setup pools are closed again).
    # bufs=3: o2(b) is read by the two-row-deferred final combine, so
    # phase2(b+2) must target a third buffer to avoid a WAR stall.
    psum_o = ctx.enter_context(tc.tile_pool(name="psum_o", bufs=3, space="PSUM"))

    prod = singles.tile([P, H], F32, tag="prod")  # phase-1 main-out scratch

    def emit_cb_dma(b):
        # casting DMA (SWDGE): HBM fp32 -> SBUF fp16
        cb = cpool.tile([P, T, H], FP16, tag="cb")
        cm_b = cm[b].rearrange("(t p) h -> p t h", p=P)
        tpg = T // DMA_GROUPS
        for g in range(DMA_GROUPS if "dma" not in skip else 0):
            nc.gpsimd.dma_start(
                out=cb[:, ds(g * tpg, tpg), :], in_=cm_b[:, ds(g * tpg, tpg), :]
            )
        return cb

    def emit_orow(b, o2):
        # out_row = o2 * beta + alpha*h_t[b]  (exact fp32 DVE STT)
        orow = opool.tile([1, H], F32, tag="orow")
        nc.vector.scalar_tensor_tensor(
            out=orow[:],
            in0=o2[:],
            scalar=be_sb[:],
            in1=aht[:, ds(b * H, H)],
            op0=ALU.mult,
            op1=ALU.add,
        )
        nc.scalar.dma_start(out=out[b : b + 1, :], in_=orow[:])

    pending = []  # two-deep deferred (b, o2) final combines
    cbs = [emit_cb_dma(0)]  # prefetch row 0

    # ---- per-batch pipeline ----
    for b in range(B_LOC):
        if b + 1 < B_LOC:
            cbs.append(emit_cb_dma(b + 1))  # prefetch next row's DMA descs
        cb = cbs[b]

        # phase 1: scores[p, t] = sum_h cb[p,t,h] * u[h]  (fused DVE STT)
        scores = spool.tile([P, T], F32, tag="scores")
        if "p1" in skip:
            nc.vector.memset(scores[:], 1.0)
        for t in range(T if "p1" not in skip else 0):
            nc.vector.scalar_tensor_tensor(
                out=prod[:],
                in0=cb[:, t, :],
                scalar=1.0,
                in1=u_bc[b][:],
                op0=ALU.mult,
                op1=ALU.mult,
                accum_out=scores[:, t : t + 1],
            )

        # partition-local softmax pieces (m_neg = -rowmax)
        m_neg = small.tile([P, 1], F32, tag="m_neg")
        nc.vector.tensor_reduce(
            out=m_neg[:], in_=scores[:], axis=mybir.AxisListType.X, op=ALU.max,
            negate=True,
        )
        e = spool.tile([P, T], F32, tag="e")
        l = small.tile([P, 1], F32, tag="l")
        nc.scalar.activation(e[:], scores[:], ACTF.Exp, bias=m_neg[:], scale=1.0, accum_out=l[:])
        # t_p = exp(m_p - C) = exp(-m_neg - C)
        tp = small.tile([P, 1], F32, tag="tp")
        nc.scalar.activation(tp[:], m_neg[:], ACTF.Exp, bias=noff[:], scale=-1.0)

        # L on every partition via gpsimd cross-partition allreduce (the
        # PE stays out of the softmax chain entirely)
        q = small.tile([P, 1], F32, tag="q")
        nc.vector.tensor_mul(q[:], l[:], tp[:])
        Lb = small.tile([P, 1], F32, tag="Lb")
        nc.gpsimd.partition_all_reduce(
            Lb[:], q[:], channels=P, reduce_op=bass_isa.ReduceOp.add
        )
        rli = small.tile([P, 1], F32, tag="rli")
        nc.vector.reciprocal(rli[:], Lb[:])
        tp2 = small.tile([P, 1], F32, tag="tp2")
        nc.vector.tensor_mul(tp2[:], tp[:], rli[:])
        # att = e * tp / L: the exact softmax, in (0,1] -> safe in fp16
        att = spool.tile([P, T], FP16, tag="att")
        nc.vector.tensor_scalar_mul(att[:], e[:], tp2[:])

        # oldest deferred final combine: its phase-2 PE work finished two
        # rows ago, so the DVE never stalls on the PE here.
        if len(pending) == 2:
            emit_orow(*pending.pop(0))

        # phase 2: o2[0, h] = sum_{p,t} att[p,t] * cb[p,t,h]  (fp16, 1 cyc/row)
        o2 = psum_o.tile([1, H], F32, tag="o2")
        if "p2" in skip:
            nc.tensor.matmul(o2[:, 0:NHALF], att[:, 0:1], cb[:, 0, 0:NHALF], start=True, stop=True)
            nc.tensor.matmul(o2[:, NHALF:H], att[:, 0:1], cb[:, 0, NHALF:H], start=True, stop=True)
        for t in range(T if "p2" not in skip else 0):
            for nh in range(2):
                nc.tensor.matmul(
                    o2[:, ds(nh * NHALF, NHALF)],
                    att[:, t : t + 1],
                    cb[:, t, ds(nh * NHALF, NHALF)],
                    start=(t == 0),
                    stop=(t == T - 1),
                )
        pending.append((b, o2))

    for p in pending:
        emit_orow(*p)


def build_bass(n_repeat=1, skip=()):
    nc = bacc.Bacc("TRN2", target_bir_lowering=False, debug=False, num_devices=N_CORES)
    ht = nc.dram_tensor("h_t", [B_LOC, H], F32, kind="ExternalInput")
    cm = nc.dram_tensor("cntx_matrix", [B_LOC, S, H], F32, kind="ExternalInput")
    w = nc.dram_tensor("W", [H, H], F32, kind="ExternalInput")
    al = nc.dram_tensor("alpha", [1], F32, kind="ExternalInput")
    be = nc.dram_tensor("beta", [1], F32, kind="ExternalInput")
    out = nc.dram_tensor("out", [B_LOC, H], F32, kind="ExternalOutput")
    with tile.TileContext(nc) as tc:
        for _ in range(n_repeat):
            with ExitStack() as ctx:
                _emit(ctx, tc, nc, ht, cm, w, al, be, out, skip=skip)
    nc.compile()
    return nc


def _shard_inputs(inputs):
    h_t = np.ascontiguousarray(np.asarray(inputs["h_t"], dtype=np.float32))
    cm = np.ascontiguousarray(np.asarray(inputs["cntx_matrix"], dtype=np.float32))
    w = np.ascontiguousarray(np.asarray(inputs["W"], dtype=np.float32))
    al = np.ascontiguousarray(np.asarray(inputs["alpha"], dtype=np.float32))
    be = np.ascontiguousarray(np.asarray(inputs["beta"], dtype=np.float32))
    in_maps = []
    for c in range(N_CORES):
        sl = slice(c * B_LOC, (c + 1) * B_LOC)
        in_maps.append(
            {
                "h_t": h_t[sl],
                "cntx_matrix": cm[sl],
                "W": w,
                "alpha": al,
                "beta": be,
            }
        )
    return in_maps


def kernel(**inputs) -> np.ndarray:
    nc = build_bass()
    in_maps = _shard_inputs(inputs)
    res = run_bass_kernel_spmd(nc, in_maps, core_ids=list(range(N_CORES)))
    return np.concatenate([r["out"] for r in res.results], axis=0).astype(np.float32)


if __name__ == "__main__":
    # quick single-core sim check against numpy
    from concourse.bass_interp import CoreSim

    rng = np.random.default_rng(0)
    h_t = rng.standard_normal((B_LOC, H), dtype=np.float32)
    cm = rng.standard_normal((B_LOC, S, H), dtype=np.float32)
    w = rng.uniform(-0.05, 0.05, size=(H, H)).astype(np.float32)
    al = np.array([1.3], dtype=np.float32)
    be = np.array([0.7], dtype=np.float32)

    nc = build_bass()
    sim = CoreSim(nc)
    sim.tensor("h_t")[:] = h_t
    sim.tensor("cntx_matrix")[:] = cm
    sim.tensor("W")[:] = w
    sim.tensor("alpha")[:] = al
    sim.tensor("beta")[:] = be
    sim.simulate()
    got = np.asarray(sim.tensor("out"))

    u = h_t @ w
    scores = np.einsum("bsh,bh->bs", cm, u)
    sm = np.exp(scores - scores.max(axis=1, keepdims=True))
    attn = sm / sm.sum(axis=1, keepdims=True)
    cx = np.einsum("bs,bsh->bh", attn, cm)
    exp = al * h_t + be * cx
    err = np.abs(got - exp).max() / np.abs(exp).max()
    print("sim rel err:", err)
